# revision 25
# baseline (speedup 1.0000x reference)
"""Trainium2 Bass kernel for AdjacencyErrorAwareLoss.

Math (reference):
    A_fid = (d_hw == 1) * max(1 - d_error, 0)                    [128,128]
    scores[b,e] = P[b,i_e,:] @ A_fid @ P[b,j_e,:]                [B,E]
    loss = -mean_b( sum_e(w*scores) / max(sum_e w, 1e-8) )

Key transformation: scores[b,e] = S_b[i_e, j_e] where S_b = P_b @ A @ P_b^T.
Per sample: two 128^3 matmuls build S_b, then a weighted gather of E=4096
scalars from the 128x128 score matrix.

Distribution: data-parallel over B=64: 8 NeuronCores x 8 samples. On each
core, sample c is handled by GPSIMD core c (partitions 16c..16c+16).

Gather strategy (ap_gather: 8 GPSIMD cores, each processing its 16
partitions with a shared per-core index list, wrapped (s p) across the
core's partitions; measured cost is linear at ~26 ns/index -- the Q7
read-command latency, ReadOverlap=0 on TRN2 -- so the 4096 indices per
core cost ~105 us and dominate the kernel):
  - partition p = 16c+q holds a masked 16384-entry table:
    table2[p, i*128+j] = S_c[i, j] if i//8 == q else 0, so a single
    shared index idx = i*128+j returns the right value on exactly one
    partition of the group and zero on the other 15 -- no separate
    selector-mask gather or mask multiply is needed.
  - the table is built by zeroing once (invariant background), then 16
    per-q stripe DMAs from a DRAM bounce of the S matrices (SBUF DMAs
    cannot collapse partitions or vary free offsets per partition).
  - w is shipped host-permuted into gather-column order and replicated
    16x across each partition group (16 row DMAs), so the weighted
    reduction is one contiguous full-width scalar_tensor_tensor with a
    fused per-partition accumulator per gather chunk; masked zeros on
    the 15 non-owner partitions contribute nothing, and one block-ones
    matmul collapses the per-partition partials per sample at the end.
  - the repeat loop is unrolled x8 per For_i trip (For_i inserts an
    all-engine barrier per trip) with double-buffered load/matmul/bounce
    tiles, so iteration i+1's prologue can overlap iteration i's
    gathers. Iteration i's tail matmuls are DEFERRED until after
    iteration i+1's matmuls are emitted, so their part1 wait never
    head-of-line-blocks the PE queue; early loads go on the ACT HWDGE
    queue and the gather-critical stripes + tail DMAs on SP, so no
    late-waiting DMA stalls next-iteration loads (HWDGE queues are FIFO
    with sequencer-level head-of-line blocking). Steady state ~143-148
    us/iter vs ~184 us un-pipelined.

Approaches measured or priced out (see memory notes): single-chunk and
8-chunk gathers are no faster (cost is per-index); indirect_dma_start
supports only one offset per partition per instruction (row gather, 128
descriptors max); dma_gather addresses HBM at idx*256B granularity (no
scalar gather) and 256B rows for both Pi/Vj would be DMA-bound at ~94
us/NC; scatter_add/local_scatter have the same per-index read-command
floor or wrong semantics; a bf16 paired-entry (d=2) table with
double-buffering measured ~26 us SLOWER back-to-back despite halved
table bytes; DVE-only tails via DRAM-bounce transposes blocked the
stripe DMAs and measured worse.
"""

import numpy as np

B, NL, NP, E = 64, 128, 128, 4096
N_CORES = 8
BPC = B // N_CORES  # samples per NeuronCore


def _patch_tile_drain():
    """This toolchain's walrus rejects >1 sem wait on a Drain; split the
    kernel-tail drain into one drain per pending semaphore."""
    import concourse.tile as tile
    from concourse.vector_clock import ScopedClock, VectorClock

    def _drain_and_barrier_split(self, tick_clock, wait_clock):
        nc = self.nc
        gc = tick_clock.global_clock  # VectorClock
        n = len(gc)
        for p in [i for i in range(n) if gc[i] > 0]:
            vec = VectorClock([gc[i] if i == p else 0 for i in range(n)])
            drain_inst = nc.sync.drain()
            wait_clock.add_sem_waits(drain_inst.ins, ScopedClock({None: vec}))
        nc.all_engine_barrier()
        assert self.sems is not None
        popped = nc._tile_sem_poison_stack.pop()
        assert popped is self._sem_poison
        nc.clear_and_free_semaphores(list(self.sems.allocated().values()))
        nc.all_engine_barrier()

    tile.TileContext._drain_and_barrier = _drain_and_barrier_split


def _split_multi_waits(nc, mybir):
    """Walrus codegen accepts at most one sem wait per instruction ("Too
    many sync wait commands"). Hoist extra waits onto preceding same-engine
    NoOps (engines execute in order, so this blocks equivalently)."""
    k = 0
    for f in nc.m.functions:
        for bb in f.blocks:
            insts = list(bb.instructions)
            out = []
            changed = False
            for ins in insts:
                si = ins.sync_info
                waits = list(si.on_wait) if si is not None and si.on_wait else []
                if len(waits) > 1:
                    changed = True
                    for w in waits[:-1]:
                        nop = mybir.InstNoOp(name=f"xw-{k}", ins=[], outs=[])
                        k += 1
                        nop.engine = ins.engine
                        nop.sync_info = mybir.SyncInfo(on_wait=[w], on_update=[])
                        nc.register_instruction(nop)
                        out.append(nop)
                    ins.sync_info = mybir.SyncInfo(
                        on_wait=[waits[-1]], on_update=list(si.on_update or [])
                    )
                out.append(ins)
            if changed:
                bb.instructions = out


def build_nc(repeat: int = 1, stage: str = "full"):
    """Build the Bass module (single-core SPMD program, run on 8 cores).

    repeat > 1 wraps the body in a hardware loop for timing runs.
    stage in ("loads", "mm", "gather", "full") truncates the body for
    cost bisection.
    """
    import concourse.bass as bass
    import concourse.mybir as mybir
    import concourse.tile as tile
    from concourse import library_config

    _patch_tile_drain()

    AL = mybir.AluOpType
    f32 = mybir.dt.float32
    i32 = mybir.dt.int32
    i16 = mybir.dt.int16

    nc = bass.Bass(detect_race_conditions=False)

    p_d = nc.dram_tensor("p", [BPC, NL, NP], f32, kind="ExternalInput")
    ep_d = nc.dram_tensor("ep", [BPC, E, 4], i32, kind="ExternalInput")
    w_d = nc.dram_tensor("w", [BPC, E], f32, kind="ExternalInput")
    derr_d = nc.dram_tensor("derr", [NP, NP], f32, kind="ExternalInput")
    dhw_d = nc.dram_tensor("dhw", [NP, NP], i32, kind="ExternalInput")
    out_d = nc.dram_tensor("out", [1, 1], f32, kind="ExternalOutput")

    # NEFF-embedded constants
    blockones_np = np.zeros((128, BPC), dtype=np.float32)
    for c in range(BPC):
        blockones_np[16 * c:16 * (c + 1), c] = 1.0
    blockones_d = nc.inline_tensor(blockones_np, name="blockones")
    ones_d = nc.inline_tensor(np.ones((128, 1), dtype=np.float32), name="ones128")
    ident_d = nc.inline_tensor(np.eye(128, dtype=np.float32), name="ident128")


    # gather chunk boundaries in 512-column blocks: tapering chunks so the
    # last gather (and the reduction tail it exposes) is small
    CHUNKS = [(0, 2), (2, 4), (4, 6), (6, 7), (7, 8)]

    with tile.TileContext(nc) as tc:
        with (
            tc.tile_pool(name="persist", bufs=1) as persist,
            tc.tile_pool(name="pp", bufs=2, space="PSUM") as pp,
            tc.tile_pool(name="vall", bufs=1, space="PSUM") as vallp,
            tc.tile_pool(name="pred", bufs=2, space="PSUM") as pred,
            tc.tile_pool(name="sdram", bufs=2, space="DRAM") as sdram,
        ):
            nc.gpsimd.load_library(library_config.ap_gather)

            # ---- persistent tiles (shared across pipeline stages)
            blockones = persist.tile([128, BPC], f32)
            ones128 = persist.tile([128, 1], f32)
            ident = persist.tile([128, 128], f32)
            nc.sync.dma_start(blockones[:], blockones_d[:])
            nc.sync.dma_start(ones128[:], ones_d[:])
            nc.sync.dma_start(ident[:], ident_d[:])

            derr = persist.tile([128, 128], f32)
            dhw = persist.tile([128, 128], i32)
            nc.sync.dma_start(derr[:], derr_d[:])
            nc.sync.dma_start(dhw[:], dhw_d[:])

            # masked gather table: [p, i*128+j] = S[i,j] if i//8 == p%16 else 0
            table2 = persist.tile([128, 16384], f32)
            afid = persist.tile([128, 128], f32)
            relu_e = persist.tile([128, 128], f32)
            mask_e = persist.tile([128, 128], f32)
            # one tile per gather chunk so chunk i+1's gather write cannot
            # false-serialize against chunk i's reduction reads
            val2s = [persist.tile([128, (b - a) * 512, ], f32, name=f"val2_{i}")
                     for i, (a, b) in enumerate(CHUNKS)]

            # double-buffered tiles: iteration i+1's loads/matmuls/bounce
            # overlap iteration i's gathers in the repeat loop
            NSET = 2
            sets = []
            for s in range(NSET):
                bs = {}
                bs["pall"] = persist.tile([128, BPC, 128], f32, name=f"pall{s}")
                bs["pt_all"] = persist.tile([128, BPC, 128], f32, name=f"pt{s}")
                bs["v_sb"] = persist.tile([128, BPC, 128], f32, name=f"v{s}")
                bs["s_all"] = persist.tile([128, BPC, 128], f32, name=f"s{s}")
                bs["epi"] = persist.tile([128, 256, 4], i32, name=f"epi{s}")
                bs["idx16"] = persist.tile([128, 256], i16, name=f"idx{s}")
                bs["t1"] = persist.tile([128, 256], i32, name=f"t1{s}")
                bs["w_nat"] = persist.tile([BPC, E], f32, name=f"w{s}")
                bs["w_g"] = persist.tile([128, E], f32, name=f"wg{s}")
                bs["scr2"] = persist.tile([128, 1024], f32, name=f"scr{s}")
                bs["part1"] = persist.tile([128, 1], f32, name=f"pp1{s}")
                bs["zaccs"] = persist.tile([128, 8], f32, name=f"za{s}")
                bs["ws8"] = persist.tile([BPC, 1], f32, name=f"ws{s}")
                bs["zdiv"] = persist.tile([BPC, 1], f32, name=f"zd{s}")
                bs["res"] = persist.tile([1, 1], f32, name=f"res{s}")
                sets.append(bs)

            # zero the masked table once (stripes are rewritten in place every
            # iteration; the zero background is invariant)
            nc.vector.memset(table2[:, 0:8192], 0.0)
            nc.scalar.memzero(table2[:, 8192:16384])

            def body(_it=0):
                bs = sets[_it % NSET]
                pall = bs["pall"]
                pt_all = bs["pt_all"]
                v_sb = bs["v_sb"]
                s_all = bs["s_all"]
                epi = bs["epi"]
                idx16 = bs["idx16"]
                t1 = bs["t1"]
                w_nat = bs["w_nat"]
                w_g = bs["w_g"]
                scr2 = bs["scr2"]
                zaccs = bs["zaccs"]
                part1 = bs["part1"]
                ws8 = bs["ws8"]
                res = bs["res"]
                # ---- P first: it feeds the transpose->mm1->mm2 PE chain,
                # which is the longest pre-gather dependency path
                p_src = bass.AP(
                    tensor=p_d, offset=0,
                    ap=[[128, 128], [NL * NP, BPC], [1, 128]],
                )
                nc.scalar.dma_start(pall[:], p_src)

                # ---- A_fid = (dhw == 1) * relu(1 - derr)
                nc.scalar.activation(
                    relu_e[:], derr[:],
                    mybir.ActivationFunctionType.Relu, bias=1.0, scale=-1.0,
                )
                nc.vector.tensor_scalar(
                    out=mask_e[:], in0=dhw[:], scalar1=1, scalar2=None,
                    op0=AL.is_equal,
                )
                nc.vector.tensor_tensor(
                    out=afid[:], in0=relu_e[:], in1=mask_e[:], op=AL.mult,
                )

                # ---- edge pairs, contiguous: partition 16c+r holds edges
                # [256r, 256r+256) of sample c; gather position k of sample c
                # is edge 256*(k%16) + k//16
                ep_src = bass.AP(
                    tensor=ep_d, offset=0,
                    ap=[[1024, 128], [4, 256], [1, 4]],
                )
                nc.scalar.dma_start(epi[:], ep_src)
                # idx = i*128 + j  (i = int32 word 0, j = word 2)
                nc.vector.scalar_tensor_tensor(
                    out=t1[:], in0=epi[:, :, 0], scalar=128,
                    in1=epi[:, :, 2], op0=AL.mult, op1=AL.add,
                )
                nc.vector.tensor_copy(idx16[:], t1[:])

                # ---- w natural (permuted) layout; wsum computed early
                nc.scalar.dma_start(w_nat[:], w_d[:])
                # w_g[16c+q, :] = w[c, :]: 16x row replication so the
                # weighted reduction is one full-width stt over the masked
                # values (w is already host-permuted to gather-column order)
                for q in range(16):
                    nc.scalar.dma_start(w_g[q::16, :], w_d[:])
                nc.vector.tensor_reduce(
                    out=ws8[:], in_=w_nat[:], axis=mybir.AxisListType.X, op=AL.add,
                )
                nc.vector.tensor_scalar(
                    out=ws8[:], in0=ws8[:], scalar1=1e-8, scalar2=None, op0=AL.max,
                )
                nc.vector.reciprocal(ws8[:], ws8[:])

                if stage == "loads":
                    nc.vector.memset(res[:], 0.0)
                    nc.sync.dma_start(out_d[:], res[:])
                    return

                # ---- per-sample transposes, then batched V = mm(A, P^T)
                for c in range(BPC):
                    pt_ps = pp.tile([128, 128], f32, tag="ptps")
                    nc.tensor.transpose(pt_ps[:], pall[:, c, :], ident[:])
                    nc.scalar.copy(pt_all[:, c, :], pt_ps[:])

                v_ps = vallp.tile([128, BPC, 128], f32)
                nc.tensor.matmul(
                    v_ps[:].rearrange("p a b -> p (a b)")[:, 0:512],
                    lhsT=afid[:],
                    rhs=pt_all[:].rearrange("p a b -> p (a b)")[:, 0:512],
                    start=True, stop=True,
                )
                nc.tensor.matmul(
                    v_ps[:].rearrange("p a b -> p (a b)")[:, 512:1024],
                    lhsT=afid[:],
                    rhs=pt_all[:].rearrange("p a b -> p (a b)")[:, 512:1024],
                    start=True, stop=True,
                )
                nc.vector.tensor_copy(v_sb[:], v_ps[:])

                # ---- S_c = V_c^T(as lhsT) @ P_c^T ; copy to s_all; bounce
                # (per-sample DRAM writes overlap the remaining matmuls)
                s_dr = sdram.tile([128, BPC, 128], f32, tag="sdram")
                for c in range(BPC):
                    s_ps = pp.tile([128, 128], f32, tag="sps")
                    nc.tensor.matmul(
                        s_ps[:], lhsT=v_sb[:, c, :], rhs=pt_all[:, c, :],
                        start=True, stop=True,
                    )
                    nc.scalar.copy(s_all[:, c, :], s_ps[:])
                    nc.scalar.dma_start(s_dr[:, c, :], s_all[:, c, :])
                # 16 stripe reads: table2[{16c+q}, 1024q : 1024q+1024] =
                #   S_c rows [8q, 8q+8)
                for q in range(16):
                    dst = table2[q::16, 1024 * q:1024 * (q + 1)]
                    nc.sync.dma_start(
                        dst,
                        s_dr[:].rearrange("l c x -> c l x")[:, 8 * q:8 * q + 8, :],
                    )

                if stage == "mm":
                    nc.vector.memset(res[:], 0.0)
                    nc.sync.dma_start(out_d[:], res[:])
                    return

                # ---- gather + reduction, chunked: the PE/DVE reduction of
                # chunk ch overlaps the GPSIMD gather of chunk ch+1
                for ch, (a, b) in enumerate(CHUNKS):
                    ss = slice(32 * a, 32 * b)
                    val2 = val2s[ch]
                    nc.gpsimd.ap_gather(
                        out_ap=val2[:].unsqueeze(2),
                        in_ap=table2[:].unsqueeze(2),
                        idxs_ap=idx16[:, ss],
                        channels=128, num_elems=16384, d=1,
                        num_idxs=(b - a) * 512,
                    )
                    if stage == "gonly":
                        continue
                    # weighted partial sums: one full-width stt per chunk;
                    # masked zeros on 15/16 partitions contribute nothing, so
                    # per-partition accumulators sum to the per-sample total.
                    # gather column k is edge 256*(k%16) + k//16, so in1 views
                    # w_g as [p, hi, lo] with lo innermost at stride 256
                    ncol = (b - a) * 512
                    nc.vector.scalar_tensor_tensor(
                        out=scr2[:, 0:ncol],
                        in0=val2[:],
                        scalar=0.0, in1=w_g[:, 512 * a:512 * b],
                        op0=AL.add, op1=AL.mult,
                        accum_out=zaccs[:, ch:ch + 1],
                    )

                if stage == "gonly":
                    nc.vector.tensor_copy(res[:], val2s[0][0:1, 0:1])
                    nc.sync.dma_start(out_d[:], res[:])
                    return

                nc.vector.tensor_reduce(
                    out=part1[:], in_=zaccs[:, 0:len(CHUNKS)],
                    axis=mybir.AxisListType.X, op=AL.add,
                )

                if stage == "gather":
                    nc.vector.memset(res[:], 0.0)
                    nc.sync.dma_start(out_d[:], res[:])
                    return

            def body_tail(_it=0):
                # finals for iteration _it, emitted AFTER iteration _it+1's
                # matmuls so the part1 wait never head-of-line-blocks the PE
                # queue: collapse 16-partition groups, divide, sum
                bs = sets[_it % NSET]
                y_ps = pred.tile([BPC, 1], f32, tag="y")
                nc.tensor.matmul(
                    y_ps[:], lhsT=blockones[:], rhs=bs["part1"][:],
                    start=True, stop=True,
                )
                nc.vector.tensor_tensor(
                    out=bs["zdiv"][:], in0=y_ps[:], in1=bs["ws8"][:],
                    op=AL.mult,
                )
                zz_ps = pred.tile([1, 1], f32, tag="y")
                nc.tensor.matmul(
                    zz_ps[:], lhsT=bs["zdiv"][:], rhs=ones128[0:BPC, :],
                    start=True, stop=True,
                )
                nc.vector.tensor_copy(bs["res"][:], zz_ps[:])
                nc.vector.tensor_scalar_mul(bs["res"][:], bs["res"][:],
                                            -1.0 / B)
                nc.sync.dma_start(out_d[:], bs["res"][:])

            def emit_seq(n):
                # main(0), main(1), tail(0), main(2), tail(1), ... tail(n-1)
                for it in range(n):
                    body(it)
                    if stage == "full" and it > 0:
                        body_tail(it - 1)
                if stage == "full":
                    body_tail(n - 1)

            if repeat == 1:
                body()
                if stage == "full":
                    body_tail(0)
            elif repeat <= 8:
                # flat unroll (no HW loop) -- lets TimelineSim run it
                emit_seq(repeat)
            else:
                # For_i inserts an all-engine barrier per trip; unroll x8
                # bodies per trip (alternating buffer sets) so the barrier
                # and pipeline ramp amortize over 8 iterations
                UN = 8
                with tc.For_i(0, repeat // UN, 1):
                    emit_seq(UN)
                if repeat % UN:
                    emit_seq(repeat % UN)


    _split_multi_waits(nc, mybir)
    # Populate .instr bytes for extended-inst InstISA subclasses (ap_gather);
    # without this the NEFF compiler sees empty .instr -> "ISA wrong length".
    mybir.codegen_inst_isa_subclasses(nc)
    return nc


def _shard_inputs(P, d_error, edge_weights, d_hw, edge_pairs):
    ep32 = edge_pairs.astype(np.int64, copy=False).view(np.int32).reshape(B, E, 4)
    derr = np.ascontiguousarray(d_error, dtype=np.float32)
    dhw = np.ascontiguousarray(d_hw, dtype=np.int32)
    # gather-column order: column k of the gathered values is edge
    # 256*(k%16) + k//16, so ship w permuted to make the device-side
    # weighted reduction a contiguous packed view (sums are order-invariant)
    k = np.arange(E)
    perm = 256 * (k % 16) + k // 16
    w_perm = np.ascontiguousarray(edge_weights[:, perm], dtype=np.float32)
    in_maps = []
    for core in range(N_CORES):
        s = slice(BPC * core, BPC * (core + 1))
        in_maps.append({
            "p": np.ascontiguousarray(P[s], dtype=np.float32),
            "ep": np.ascontiguousarray(ep32[s]),
            "w": w_perm[s],
            "derr": derr,
            "dhw": dhw,
        })
    return in_maps


def kernel(P, d_error, edge_weights, d_hw, edge_pairs):
    from concourse.bass_utils import run_bass_kernel_spmd

    nc = build_nc()
    in_maps = _shard_inputs(P, d_error, edge_weights, d_hw, edge_pairs)
    res = run_bass_kernel_spmd(nc, in_maps, core_ids=list(range(N_CORES)))
    total = np.float32(0.0)
    for r in res.results:
        total += np.float32(r["out"][0, 0])
    return np.float32(total)

